# revision 27
# baseline (speedup 1.0000x reference)
"""KBLN scorer kernel for 8 TRN2 NeuronCores — rank-R Gaussian basis.

out[b,e] = sum_f w[b,f] * exp(-(u[b,f] - t[e,f])^2),  t = lit/sqrt(var),
u = (head_lit - c)/sqrt(var).

Key idea: the Gaussian kernel K(u,t) = exp(-(u-t)^2) is numerically
low-rank over the data range. Per feature f we pick R fixed centers
g[f,k] and fit, on the host, coefficients C[f,k,b] so that

    K(u[b,f], t) ~= sum_k C[f,k,b] * exp(-gamma (t - g[f,k])^2).

Then out[b,e] = sum_{f,k} (C[f,k,b] w[b,f]) * phi[f,k](t[e,f]) — a single
contraction over F*R lanes that PSUM accumulates across R/2
partition-chunks of 128. On device each basis function costs exactly ONE
instruction: the scalar engine's Derivative_Erf activation is
(2/sqrt(pi)) * exp(-x^2), so with x = sqrt(gamma)*t - sqrt(gamma)*g
(per-partition bias) it evaluates the Gaussian directly; the 2/sqrt(pi)
and gamma fold into the host-fit coefficients. No vector-engine build
ops at all — ACT streams basis tiles straight into float32r matmuls.

Entities are sharded 8 ways (spec sharding_hint); c/var/nf_weights and
the head/rel batch are folded into the replicated coefficients.
"""

import numpy as np

import concourse.bass as bass
import concourse.tile as tile
from concourse import mybir
from concourse.bass_utils import run_bass_kernel_spmd
from concourse.tile import ScopedClock

E = 50000
F = 64
B = 64
NCORES = 8
E_SH = 6272          # padded shard: 8 * 6272 = 50176
E_PAD = E_SH * NCORES
# Laddered entity blocks (size, psum banks): small first block so ACT
# starts early (short DMA fill), small last block so the psum-copy /
# out-DMA / drain tail is short; wide middle blocks amortize the ACT
# per-instruction SBUF-access penalty. Matmul width = size/banks must be
# in [256, 512] (f32r 1 cycle/row needs >=256; PSUM bank holds 512 f32).
SBLKS = ((392, 1), (1568, 4), (2352, 6), (1568, 4), (392, 1))
assert sum(s for s, _ in SBLKS) == E_SH
assert all(s % n == 0 and 256 <= s // n <= 512 for s, n in SBLKS)
R = 10               # basis rank per feature
R2 = R // 2          # partition-chunks (2 basis fns stacked per 128 lanes)
GAMMA = 0.7

f32 = mybir.dt.float32
f32r = mybir.dt.float32r


def _drain_and_barrier_split(self, tick_clock, wait_clock):
    # This walrus build accepts only one sync-wait per TPB_CTRL Drain;
    # spread the tail-drain waits across drains distributed over the five
    # engine queues so they resolve in parallel rather than as one serial
    # chain on SP.
    drain_inst = self.nc.sync.drain()
    wait_clock.add_sem_waits(drain_inst.ins, ScopedClock({None: tick_clock.global_clock}))
    si = drain_inst.ins.sync_info
    waits = list(si.on_wait or [])
    if len(waits) > 1:
        si.on_wait = waits[:1]
        engines = [
            self.nc.sync,
            self.nc.vector,
            self.nc.scalar,
            self.nc.tensor,
            self.nc.gpsimd,
        ]
        for i, w in enumerate(waits[1:]):
            extra = engines[i % len(engines)].drain()
            esi = extra.ins.sync_info
            if esi is None:
                from bass_rust import SyncInfo

                extra.ins.sync_info = SyncInfo(on_wait=[w], on_update=[])
            else:
                esi.on_wait = [w]
    self.nc.all_engine_barrier()
    popped = self.nc._tile_sem_poison_stack.pop()
    assert popped is self._sem_poison
    self.nc.clear_and_free_semaphores(list(self.sems.allocated().values()))
    self.nc.all_engine_barrier()


tile.TileContext._drain_and_barrier = _drain_and_barrier_split


def _split_excess_waits(nc, maxw=1):
    """This walrus build rejects instructions carrying more than one
    sync-wait. Hoist excess waits onto NOPs inserted just before the
    instruction on the same engine queue (same blocking semantics)."""
    from bass_rust import SyncInfo

    for f in nc.m.functions:
        for bb in f.blocks:
            new = []
            changed = False
            for inst in bb.instructions:
                si = inst.sync_info
                waits = list(si.on_wait) if si is not None and si.on_wait else []
                if len(waits) > maxw:
                    changed = True
                    extra, keep = waits[:-maxw], waits[-maxw:]
                    for i in range(0, len(extra), maxw):
                        nop = mybir.InstNoOp(
                            name=f"{inst.name}.w{i}",
                            engine=inst.engine,
                            ins=[],
                            outs=[],
                            sync_info=SyncInfo(
                                on_wait=extra[i : i + maxw], on_update=[]
                            ),
                        )
                        new.append(nop)
                    si.on_wait = keep
                new.append(inst)
            if changed:
                try:
                    bb.instructions[:] = new
                except TypeError:
                    bb.instructions = new


_NC_CACHE = None


def build_nc():
    global _NC_CACHE
    if _NC_CACHE is not None:
        return _NC_CACHE
    nc = bass.Bass(trn_type="TRN2")
    t2 = nc.dram_tensor("t2", [128, E_SH], f32, kind="ExternalInput")
    biases = nc.dram_tensor("biases", [128, R2], f32, kind="ExternalInput")
    cw = nc.dram_tensor("cw", [128, R2 * B], f32r, kind="ExternalInput")
    out = nc.dram_tensor("out", [B, E_SH], f32, kind="ExternalOutput")

    with tile.TileContext(nc) as tc:
        with (
            tc.tile_pool(name="sb", bufs=6) as sbpool,
            tc.tile_pool(name="hp", bufs=8) as hpool,
            tc.tile_pool(name="ps", bufs=8, space="PSUM") as pspool,
        ):
            singles = litpool = sbpool
            # Warm the ACT function table while the first DMAs are in
            # flight: a dummy Derivative_Erf on a memset tile triggers the
            # (1.3us) table load off the critical path.
            warm = singles.tile([128, 1], f32, tag="warm", name="warm")
            nc.vector.memset(warm, 0.0)
            warm_o = singles.tile([128, 1], f32, tag="warm_o", name="warm_o")
            nc.scalar.activation(
                out=warm_o,
                in_=warm,
                func=mybir.ActivationFunctionType.Derivative_Erf,
                bias=warm[:, 0:1],
                scale=1.0,
            )

            bi_sb = singles.tile([128, R2], f32, tag="bi", name="bi_sb")
            nc.sync.dma_start(out=bi_sb, in_=biases.ap())

            # first (small) block's t2 is DMA'd before cw so ACT starts sooner;
            # cw is only needed once the first matmul issues.
            t2_first = litpool.tile([128, SBLKS[0][0]], f32, tag="t2f", name="t2_first")
            nc.sync.dma_start(out=t2_first, in_=t2.ap()[:, 0 : SBLKS[0][0]])

            # second block's t2 also goes ahead of cw: its transfer must
            # finish exactly when the first block's five ACTs do, while cw
            # is only needed by the first matmul (PE has slack to catch up).
            t2_second = litpool.tile([128, SBLKS[1][0]], f32, tag="t2s", name="t2_second")
            nc.sync.dma_start(
                out=t2_second, in_=t2.ap()[:, SBLKS[0][0] : SBLKS[0][0] + SBLKS[1][0]]
            )

            cw_sb = singles.tile([128, R2 * B], f32r, tag="cw", name="cw_sb")
            nc.sync.dma_start(out=cw_sb, in_=cw.ap())

            base = 0
            for s, (sblk, nmm) in enumerate(SBLKS):
                mmw = sblk // nmm
                if s == 0:
                    t2_s = t2_first
                elif s == 1:
                    t2_s = t2_second
                else:
                    t2_s = litpool.tile([128, sblk], f32, tag="t2", name=f"t2_{s}")
                    nc.sync.dma_start(out=t2_s, in_=t2.ap()[:, base : base + sblk])

                psums = [
                    pspool.tile([B, mmw], f32, tag="ps", name=f"ps_{s}_{m}")
                    for m in range(nmm)
                ]
                for j in range(R2):
                    h = hpool.tile([128, sblk], f32r, tag="h", name=f"h_{s}_{j}")
                    nc.scalar.activation(
                        out=h,
                        in_=t2_s,
                        func=mybir.ActivationFunctionType.Derivative_Erf,
                        bias=bi_sb[:, j : j + 1],
                        scale=1.0,
                    )
                    for m in range(nmm):
                        nc.tensor.matmul(
                            psums[m],
                            lhsT=cw_sb[:, j * B : (j + 1) * B],
                            rhs=h[:, m * mmw : (m + 1) * mmw],
                            start=(j == 0),
                            stop=(j == R2 - 1),
                        )
                        if j == R2 - 1:
                            osl = slice(base + m * mmw, base + (m + 1) * mmw)
                            osb = hpool.tile([B, mmw], f32, tag="o", name=f"o_{s}_{m}")
                            nc.vector.tensor_copy(osb, psums[m])
                            nc.sync.dma_start(out=out.ap()[:, osl], in_=osb)
                base += sblk
    _split_excess_waits(nc)
    _NC_CACHE = nc
    return nc


def _fit_basis(t, u, w):
    """Per-feature Gaussian-basis fit.

    t: [E, F] normalized entity coords; u: [B, F] normalized head coords;
    w: [B, F] per-(batch,feature) weights.
    Returns per-lane biases [128, R2] and cw [128, R2*B].
    """
    lo = t.min(axis=0)                      # [F]
    hi = t.max(axis=0)
    # centers [F, R]: blend of data quantiles (density) and uniform (coverage)
    qs = np.quantile(t[::5], np.linspace(0.003, 0.997, R), axis=0).T   # [F, R]
    gu = lo[:, None] + (hi - lo)[:, None] * (np.arange(R) / (R - 1))[None, :]
    g = 0.7 * qs + 0.3 * gu

    # fit at (subsampled) actual entity coords per feature [F, G]
    td = t[::21].T                          # [F, G]
    # target kernel at fit points: [F, G, B]
    Kd = np.exp(-((u.T[:, None, :] - td[:, :, None]) ** 2))
    I_r = 1e-8 * np.eye(R)[None]

    def _solve(g):
        Pd = np.exp(-GAMMA * (td[:, :, None] - g[:, None, :]) ** 2)
        A = np.einsum("fgr,fgs->frs", Pd, Pd) + I_r
        Y = np.einsum("fgr,fgb->frb", Pd, Kd)
        C = np.linalg.solve(A, Y)           # [F, R, B]
        Rs = np.einsum("fgr,frb->fgb", Pd, C) - Kd
        return Pd, C, Rs

    Pd, C, Rs = _solve(g)
    # Gauss-Newton refinement of center positions (3 iters); the step
    # computation only steers center placement, so float32 is plenty.
    for _ in range(3):
        D = (2 * GAMMA * (td[:, :, None] - g[:, None, :]) * Pd).astype(np.float32)
        C32 = C.astype(np.float32)
        Rs32 = Rs.astype(np.float32)
        JTJ = np.einsum("fgk,fkb,fgl,flb->fkl", D, C32, D, C32, optimize=True)
        JTr = np.einsum("fgk,fkb,fgb->fk", D, C32, Rs32, optimize=True)
        dg = np.linalg.solve(
            JTJ.astype(np.float64) + 1e-6 * np.eye(R)[None], -JTr[..., None].astype(np.float64)
        )[..., 0]
        g = g + np.clip(dg, -0.3, 0.3)
        Pd, C, Rs = _solve(g)

    # Device computes phi = Derivative_Erf(t' + bias) = 2/sqrt(pi) *
    # exp(-(t'-g')^2) with t' = sqrt(gamma)*t, g' = sqrt(gamma)*g; the
    # 2/sqrt(pi) folds into the coefficients.
    sg = np.sqrt(GAMMA)
    # lane layout: lane<64 -> (f=lane, k=2j); lane>=64 -> (f=lane-64, k=2j+1)
    g_l = np.empty((128, R2))
    c_l = np.empty((128, R2, B))
    g_l[:F] = sg * g[:, 0::2]
    g_l[F:] = sg * g[:, 1::2]
    c_l[:F] = C[:, 0::2, :]
    c_l[F:] = C[:, 1::2, :]

    biases = (-g_l).astype(np.float32)
    wf = np.concatenate([w.T, w.T], axis=0)  # [128, B]
    cw = (np.sqrt(np.pi) / 2.0 * c_l * wf[:, None, :]).reshape(128, R2 * B)
    return biases, cw.astype(np.float32)


def _host_prep(numerical_literals, c, var, nf_weights, head_ids, rel_ids):
    lit = np.asarray(numerical_literals, dtype=np.float64)
    c64 = np.asarray(c, dtype=np.float64)
    var64 = np.asarray(var, dtype=np.float64)
    w = np.asarray(nf_weights, dtype=np.float64)[np.asarray(rel_ids)]
    a = lit[np.asarray(head_ids)] - c64          # [B, F]

    sv = np.sqrt(var64)
    t = lit / sv                                 # [E, F]
    u = a / sv                                   # [B, F]

    biases, cw = _fit_basis(t, u, w)

    tp = np.zeros((E_PAD, F), dtype=np.float32)
    tp[:E] = (np.sqrt(GAMMA) * t).astype(np.float32)

    in_maps = []
    for i in range(NCORES):
        sh = tp[i * E_SH : (i + 1) * E_SH].T     # [F, E_SH]
        t2 = np.ascontiguousarray(np.concatenate([sh, sh], axis=0))
        in_maps.append({"t2": t2, "biases": biases, "cw": cw})
    return in_maps


def kernel(numerical_literals, c, var, nf_weights, head_ids, rel_ids):
    nc = build_nc()
    in_maps = _host_prep(numerical_literals, c, var, nf_weights, head_ids, rel_ids)
    res = run_bass_kernel_spmd(nc, in_maps, core_ids=list(range(NCORES)))
    out = np.concatenate([res.results[i]["out"] for i in range(NCORES)], axis=1)
    return np.ascontiguousarray(out[:, :E])
